# revision 6
# baseline (speedup 1.0000x reference)
"""Causal GQA self-attention (B=2, T=2048, C=1024, 16 q-heads / 4 kv-heads,
RoPE, causal softmax, output projection) on 8 Trainium2 NeuronCores.

Sharding: core c = b*4 + hg handles batch b (2-way data parallel) and
head-group hg (4-way tensor parallel: 4 q-heads + their 1 shared kv head).
W_qkv is column-sharded, W_proj row-sharded; each core emits a partial
projection [2048, 1024] and the host sums the 4 partials per batch.

Device pipeline per core (all bf16 matmul inputs, fp32 PSUM accumulate):
  1. qkv = x @ W_qkv_shard, natural layout [token, 384+1] (q|k|v|ones)
  2. RoPE on q,k in natural layout (head-dim pairs pre-permuted even|odd
     on the host so rotation is two contiguous 32-blocks per head)
  3. DMA-transpose q,k -> qT [256, T], kT duplicated to partitions 0-63
     and 64-127 for 2-head row-packed score matmuls
  4. flash-style: per head, per 512-wide q chunk: S^T = k @ qT (PE,
     K=64 row-packed pairs), exp via ScalarE (scale=1/8 folded in),
     causal mask on diagonal blocks, y[q,64|den] += P^T.T @ [v|1]
  5. normalize_recip on GPSIMD (y /= den), DMA-transpose into yT
  6. out = yT.T @ W_proj_shard, DMA PSUM -> DRAM
"""

import sys

if "/opt/trn_rl_repo" not in sys.path:
    sys.path.insert(0, "/opt/trn_rl_repo")

import numpy as np
import ml_dtypes

B, T, C = 2, 2048, 1024
NH, NKV, HD = 16, 4, 64
THETA = 10000.0
NQ = NH // NKV          # q heads per core = 4
TT = T // 128           # 16 token tiles
NCHUNK = T // 512       # 4 q-chunks
BF16 = ml_dtypes.bfloat16

_CACHE = {}


def _build():
    """Build the SPMD Bass program (identical on all 8 cores)."""
    import concourse.bass as bass
    import concourse.mybir as mybir
    import concourse.tile as tile
    from concourse import bacc
    from concourse.bass import ts

    dt = mybir.dt
    AF = mybir.ActivationFunctionType

    nc = bacc.Bacc("TRN2", target_bir_lowering=False, debug=False, num_devices=8)

    xt_d = nc.declare_dram_parameter("xT", [C, T], dt.bfloat16, isOutput=False)
    w_d = nc.declare_dram_parameter("w384", [C, 384], dt.bfloat16, isOutput=False)
    wo_d = nc.declare_dram_parameter("wo", [NQ * HD, C], dt.bfloat16, isOutput=False)
    cs_d = nc.declare_dram_parameter("cs", [T, 320], dt.bfloat16, isOutput=False)
    mk_d = nc.declare_dram_parameter("masku", [128, 128], dt.bfloat16, isOutput=False)
    out_d = nc.declare_dram_parameter("out", [T, C], dt.float32, isOutput=True)

    with tile.TileContext(nc) as tc:
        from contextlib import ExitStack

        with ExitStack() as ctx:
            persist = ctx.enter_context(tc.tile_pool(name="persist", bufs=1))
            rope_tmp = ctx.enter_context(tc.tile_pool(name="rope_tmp", bufs=4))
            p_pool = ctx.enter_context(tc.tile_pool(name="p_pool", bufs=6))
            yev_pool = ctx.enter_context(tc.tile_pool(name="yev", bufs=3))
            yn_pool = ctx.enter_context(tc.tile_pool(name="yn", bufs=4))
            po_pool = ctx.enter_context(tc.tile_pool(name="po", bufs=4))
            qkv_ps_pool = ctx.enter_context(
                tc.tile_pool(name="qkv_ps", bufs=1, space="PSUM")
            )
            s_ps_pool = ctx.enter_context(
                tc.tile_pool(name="s_ps", bufs=3, space="PSUM")
            )
            y_ps_pool = ctx.enter_context(
                tc.tile_pool(name="y_ps", bufs=1, space="PSUM")
            )
            pr_ps_pool = ctx.enter_context(
                tc.tile_pool(name="pr_ps", bufs=2, space="PSUM")
            )

            # ---- persistent SBUF ----
            xt_sb = persist.tile([128, 8, T], dt.bfloat16)
            w_sb = persist.tile([128, 8, 384], dt.bfloat16)
            wo_sb = persist.tile([128, 2, C], dt.bfloat16)
            cs_sb = persist.tile([128, TT, 320], dt.bfloat16)
            mk_sb = persist.tile([128, 128], dt.bfloat16)
            qkvn = persist.tile([128, TT, 385], dt.bfloat16)
            qt_sb = persist.tile([128, 2, T], dt.bfloat16)   # [qdim, dimtile, tok]
            kt_sb = persist.tile([128, TT, 128], dt.bfloat16)  # rows 0-63 & 64-127 = kT
            ynt_sb = persist.tile([128, 2, T], dt.bfloat16)  # [ydim, dimtile, tok]

            nc.sync.dma_start(xt_sb[:], xt_d.ap().rearrange("(c p) t -> p c t", p=128))
            nc.sync.dma_start(w_sb[:], w_d.ap().rearrange("(c p) n -> p c n", p=128))
            nc.sync.dma_start(wo_sb[:], wo_d.ap().rearrange("(c p) n -> p c n", p=128))
            nc.sync.dma_start(cs_sb[:], cs_d.ap().rearrange("(n p) d -> p n d", p=128))
            nc.sync.dma_start(mk_sb[:], mk_d.ap())

            # ---- phase 1: qkv + rope, phase 2: transposes ----
            for tt in range(TT):
                ps = qkv_ps_pool.tile([128, 384], dt.float32)
                for c in range(8):
                    nc.tensor.matmul(
                        ps[:],
                        lhsT=xt_sb[:, c, ts(tt, 128)],
                        rhs=w_sb[:, c, :],
                        start=(c == 0),
                        stop=(c == 7),
                    )
                nc.scalar.copy(qkvn[:, tt, 0:384], ps[:])
                nc.vector.memset(qkvn[:, tt, 384:385], 1.0)

                base = qkvn[:, tt, 0:320].rearrange("p (h d) -> p h d", h=5)
                x1 = base[:, :, 0:32]
                x2 = base[:, :, 32:64]
                cos = cs_sb[:, tt, 0:160].rearrange("p (h d) -> p h d", h=5)
                sin = cs_sb[:, tt, 160:320].rearrange("p (h d) -> p h d", h=5)
                t1 = rope_tmp.tile([128, 5, 32], dt.bfloat16, tag="t1")
                t2 = rope_tmp.tile([128, 5, 32], dt.bfloat16, tag="t2")
                t3 = rope_tmp.tile([128, 5, 32], dt.bfloat16, tag="t3")
                t4 = rope_tmp.tile([128, 5, 32], dt.bfloat16, tag="t4")
                nc.vector.tensor_mul(t1[:], x1, cos)
                nc.vector.tensor_mul(t2[:], x2, sin)
                nc.vector.tensor_mul(t3[:], x1, sin)
                nc.vector.tensor_mul(t4[:], x2, cos)
                nc.vector.tensor_sub(x1, t1[:], t2[:])
                nc.vector.tensor_add(x2, t3[:], t4[:])

                # transposes: q dims 0-127 / 128-255, k dims (256-319) twice
                nc.sync.dma_start_transpose(
                    qt_sb[:, 0, ts(tt, 128)], qkvn[:, tt, 0:128]
                )
                nc.sync.dma_start_transpose(
                    qt_sb[:, 1, ts(tt, 128)], qkvn[:, tt, 128:256]
                )
                nc.sync.dma_start_transpose(
                    kt_sb[:, tt, :], qkvn[:, tt, 256:384]
                )
                nc.sync.dma_start(kt_sb[64:128, tt, :], kt_sb[0:64, tt, :])

            # ---- phase 3+4: attention (flash-style, head pairs row-packed)
            # fused with the output projection per 512-wide q chunk ----
            for j in range(NCHUNK):      # 512-wide q chunks
                for hp in range(2):      # head pairs (2hp, 2hp+1)
                    y_ps = [
                        y_ps_pool.tile(
                            [128, 4, 65], dt.float32, tag=f"y{u}", name=f"y_ps{u}"
                        )
                        for u in range(2)
                    ]
                    for i in range(4 * j + 4):  # k tiles
                        p_sb = []
                        for u in range(2):  # head-in-pair (partitions 64u..)
                            s_ps = s_ps_pool.tile([128, 512], dt.float32)
                            nc.tensor.matmul(
                                s_ps[:],
                                lhsT=kt_sb[ts(u, 64), i, :],
                                rhs=qt_sb[ts(u, 64), hp, ts(j, 512)],
                                start=True,
                                stop=True,
                            )
                            p_t = p_pool.tile([128, 512], dt.bfloat16)
                            nc.scalar.activation(p_t[:], s_ps[:], AF.Exp, scale=0.125)
                            off = 128 * i - 512 * j
                            if off >= 0:  # diagonal band
                                if off > 0:
                                    nc.vector.memset(p_t[:, 0:off], 0.0)
                                nc.vector.tensor_mul(
                                    p_t[:, off : off + 128],
                                    p_t[:, off : off + 128],
                                    mk_sb[:],
                                )
                            p_sb.append(p_t)
                        for u in range(2):
                            for t4 in range(4):
                                t = 4 * j + t4
                                if i > t:
                                    continue
                                # one accumulation group per psum bank:
                                # start zeroes the whole 2KB zero-region
                                nc.tensor.matmul(
                                    y_ps[u][:, t4, :],
                                    lhsT=p_sb[u][:, ts(t4, 128)],
                                    rhs=qkvn[:, i, 320:385],
                                    start=(i == 0 and t4 == 0),
                                    stop=(i == 4 * j + 3 and t4 == 3),
                                )
                    yevs = []
                    for u in range(2):
                        yev = yev_pool.tile(
                            [128, 4, 65], dt.float32, tag=f"yev{u}", name=f"yev{u}"
                        )
                        nc.vector.tensor_copy(yev[:], y_ps[u][:])
                        yevs.append(yev)
                    for t4 in range(4):
                        yn = yn_pool.tile([128, 128], dt.bfloat16)
                        for u in range(2):
                            nc.gpsimd.normalize_recip(
                                yn[:, ts(u, 64)],
                                yevs[u][:, t4, 0:64],
                                yevs[u][:, t4, 64:65],
                            )
                        nc.sync.dma_start_transpose(
                            ynt_sb[:, hp, ts(4 * j + t4, 128)], yn[:]
                        )
                # projection for this chunk's 4 token tiles (all heads ready)
                for t4 in range(4):
                    tt = 4 * j + t4
                    for nn2 in range(2):
                        ps = pr_ps_pool.tile([128, 512], dt.float32)
                        for dtile in range(2):
                            nc.tensor.matmul(
                                ps[:],
                                lhsT=ynt_sb[:, dtile, ts(tt, 128)],
                                rhs=wo_sb[:, dtile, ts(nn2, 512)],
                                start=(dtile == 0),
                                stop=(dtile == 1),
                            )
                        po = po_pool.tile([128, 512], dt.float32)
                        nc.vector.tensor_copy(po[:], ps[:])
                        nc.sync.dma_start(
                            out_d.ap()[ts(tt, 128), ts(nn2, 512)], po[:]
                        )

    nc.finalize()
    return nc


def _host_inputs(x, W_qkv, W_proj):
    """Per-core input maps (host-side sharding + layout prep)."""
    perm = np.concatenate([np.arange(0, HD, 2), np.arange(1, HD, 2)])  # even|odd
    inv = 1.0 / THETA ** (np.arange(0, HD, 2, dtype=np.float64) / HD)  # [32]
    ang = np.arange(T, dtype=np.float64)[:, None] * inv[None, :]       # [T, 32]
    cos5 = np.tile(np.cos(ang), (1, 5))
    sin5 = np.tile(np.sin(ang), (1, 5))
    cs = np.concatenate([cos5, sin5], axis=1).astype(BF16)             # [T, 320]
    masku = np.triu(np.ones((128, 128), dtype=np.float32)).astype(BF16)

    in_maps = []
    for core in range(8):
        b, hg = divmod(core, 4)
        xT = np.ascontiguousarray(x[b].T).astype(BF16)                 # [C, T]
        cols = []
        for h in range(hg * NQ, hg * NQ + NQ):
            cols.append(W_qkv[:, h * HD : (h + 1) * HD][:, perm])
        kblk = W_qkv[:, NH * HD + hg * HD : NH * HD + (hg + 1) * HD][:, perm]
        vblk = W_qkv[:, (NH + NKV) * HD + hg * HD : (NH + NKV) * HD + (hg + 1) * HD]
        w384 = np.concatenate(cols + [kblk, vblk], axis=1).astype(BF16)
        wo = np.ascontiguousarray(
            W_proj[hg * NQ * HD : (hg + 1) * NQ * HD, :]
        ).astype(BF16)
        in_maps.append(
            {"xT": xT, "w384": w384, "wo": wo, "cs": cs, "masku": masku}
        )
    return in_maps


def _run(in_maps):
    from concourse.bass_utils import run_bass_kernel_spmd

    if "nc" not in _CACHE:
        _CACHE["nc"] = _build()
    return run_bass_kernel_spmd(_CACHE["nc"], in_maps, core_ids=list(range(8)))


def kernel(x, W_qkv, W_proj):
    x = np.asarray(x, dtype=np.float32)
    W_qkv = np.asarray(W_qkv, dtype=np.float32)
    W_proj = np.asarray(W_proj, dtype=np.float32)
    res = _run(_host_inputs(x, W_qkv, W_proj))
    out = np.zeros((B, T, C), dtype=np.float32)
    for core in range(8):
        b = core // 4
        out[b] += res.results[core]["out"]
    return out


# revision 11
# speedup vs baseline: 287.8809x; 287.8809x over previous
"""Causal GQA self-attention (B=2, T=2048, C=1024, 16 q-heads / 4 kv-heads,
RoPE, causal softmax, output projection) on 8 Trainium2 NeuronCores.

Sharding: core c = b*4 + hg handles batch b (2-way data parallel) and
head-group hg (4-way tensor parallel: its 4 q-heads + their shared kv head).
W_qkv is column-sharded, W_proj row-sharded; each core emits a partial
projection [2048, 1024] and the host sums the 4 partials per batch.

Device pipeline per core (bf16 matmul inputs, fp32 PSUM accumulate):
  1. qkv = x @ W_qkv_shard, natural layout [token, 320|1|64] (q,k | ones | v)
  2. RoPE on q,k in natural layout (head-dim pairs pre-permuted even|odd on
     the host, so the rotation is two contiguous 32-blocks per head)
  3. DMA-xbar-transpose q,k -> qT [256, T]; kT duplicated to partitions 0-63
     and 64-127 (row-packed score matmuls, 2 heads concurrent in the array)
  4. flash-style per 512-wide q chunk: S^T = k @ qT (K=64), exp on ScalarE
     (1/sqrt(64) folded into the activation scale), causal masking only on
     diagonal 128-blocks, y^T[1+64, q] += [1|v]^T @ P^T (v_aug stationary)
  5. y^T row 0 is the softmax denominator: reciprocal_approx + GPSIMD
     partition_broadcast + one fused scale-evacuate multiply
  6. out = yT.T @ W_proj_shard
"""

import sys

if "/opt/trn_rl_repo" not in sys.path:
    sys.path.insert(0, "/opt/trn_rl_repo")

import numpy as np
import ml_dtypes

B, T, C = 2, 2048, 1024
NH, NKV, HD = 16, 4, 64
THETA = 10000.0
NQ = NH // NKV          # q heads per core = 4
TT = T // 128           # 16 token tiles
NCHUNK = T // 512       # 4 q-chunks
BF16 = ml_dtypes.bfloat16

_CACHE = {}


def _build():
    """Build the SPMD Bass program (identical on all 8 cores)."""
    import concourse.mybir as mybir
    import concourse.tile as tile
    from concourse import bacc
    from concourse.bass import ts
    from contextlib import ExitStack

    dt = mybir.dt
    AF = mybir.ActivationFunctionType

    nc = bacc.Bacc("TRN2", target_bir_lowering=False, debug=False, num_devices=8)

    # host pre-shuffled, partition-major inputs (contiguous per partition)
    xt_d = nc.declare_dram_parameter("xT", [128, 8 * T], dt.bfloat16, isOutput=False)
    w_d = nc.declare_dram_parameter("w384", [128, 8 * 384], dt.bfloat16, isOutput=False)
    wo_d = nc.declare_dram_parameter("wo", [128, 2 * C], dt.bfloat16, isOutput=False)
    cs_d = nc.declare_dram_parameter("cs", [128, TT * 320], dt.bfloat16, isOutput=False)
    mk_d = nc.declare_dram_parameter("masku", [128, 128], dt.bfloat16, isOutput=False)
    out_d = nc.declare_dram_parameter("out", [T, C], dt.float32, isOutput=True)

    with tile.TileContext(nc) as tc, ExitStack() as ctx:
        persist = ctx.enter_context(tc.tile_pool(name="persist", bufs=1))
        rope_tmp = ctx.enter_context(tc.tile_pool(name="rope_tmp", bufs=4))
        p_pool = ctx.enter_context(tc.tile_pool(name="p_pool", bufs=8))
        po_pool = ctx.enter_context(tc.tile_pool(name="po", bufs=4))
        ysb_pool = ctx.enter_context(tc.tile_pool(name="ysb", bufs=2))
        bc_pool = ctx.enter_context(tc.tile_pool(name="bc", bufs=2))
        yst_pool = ctx.enter_context(tc.tile_pool(name="yst", bufs=4))
        s_ps_pool = ctx.enter_context(tc.tile_pool(name="s_ps", bufs=3, space="PSUM"))
        pr_ps_pool = ctx.enter_context(tc.tile_pool(name="pr_ps", bufs=1, space="PSUM"))
        y_ps_pool = ctx.enter_context(tc.tile_pool(name="y_ps", bufs=1, space="PSUM"))

        # ---- persistent SBUF (split tiles for fine-grained scheduling) ----
        w_sb = persist.tile([128, 8, 384], dt.bfloat16)
        nc.sync.dma_start(w_sb[:], w_d.ap().rearrange("p (c n) -> p c n", c=8))
        xt_sb = []
        for jc in range(NCHUNK):
            xt = persist.tile([128, 8, 512], dt.bfloat16, name=f"xtc{jc}")
            nc.sync.dma_start(
                xt[:],
                xt_d.ap()[:, ts(jc, 8 * 512)].rearrange("p (c t) -> p c t", c=8),
            )
            xt_sb.append(xt)
            if jc == 0:
                cs_sb = persist.tile([128, TT, 320], dt.bfloat16)
                nc.sync.dma_start(
                    cs_sb[:], cs_d.ap().rearrange("p (n d) -> p n d", n=TT)
                )
                mk_sb = persist.tile([128, 128], dt.bfloat16)
                nc.sync.dma_start(mk_sb[:], mk_d.ap())
        wo_sb = persist.tile([128, 2, C], dt.bfloat16)
        nc.sync.dma_start(wo_sb[:], wo_d.ap().rearrange("p (c n) -> p c n", c=2))

        qkvn = []   # per chunk: [128, 4, 385] = q,k rot | ones | v
        qt_sb = [[None] * NCHUNK for _ in range(2)]   # [dimtile][chunk] [128,512]
        kt_sb = []  # per chunk [128, 4, 128]: kT at rows 0-63 and 64-127
        ynt = [[None] * NCHUNK for _ in range(2)]     # [dimtile][chunk] [128,512]
        for d in range(2):
            for j in range(NCHUNK):
                qt_sb[d][j] = persist.tile([128, 512], dt.bfloat16, name=f"qt{d}_{j}")
                ynt[d][j] = persist.tile([128, 512], dt.bfloat16, name=f"ynt{d}_{j}")

        # ---- phase 1+2: qkv, rope, transposes (per 4-token-tile chunk) ----
        for jc in range(NCHUNK):
            qn = persist.tile([128, 4, 385], dt.bfloat16, name=f"qkvn{jc}")
            qkvn.append(qn)
            for t4 in range(4):
                tt = 4 * jc + t4
                ps = pr_ps_pool.tile([128, 384], dt.float32, tag="t", name="qkv_ps")
                for c in range(8):
                    nc.tensor.matmul(
                        ps[:],
                        lhsT=xt_sb[jc][:, c, ts(t4, 128)],
                        rhs=w_sb[:, c, :],
                        start=(c == 0),
                        stop=(c == 7),
                    )
                nc.vector.tensor_copy(qn[:, t4, 0:320], ps[:, 0:320])
                nc.vector.tensor_copy(qn[:, t4, 321:385], ps[:, 320:384])
            nc.gpsimd.memset(qn[:, :, 320:321], 1.0)

            base = qn[:, :, 0:320].rearrange("p f (h d) -> p f h d", h=5)
            x1 = base[:, :, :, 0:32]
            x2 = base[:, :, :, 32:64]
            csc = cs_sb[:, ts(jc, 4), :]
            cos = csc[:, :, 0:160].rearrange("p f (h d) -> p f h d", h=5)
            sin = csc[:, :, 160:320].rearrange("p f (h d) -> p f h d", h=5)
            t1 = rope_tmp.tile([128, 4, 5, 32], dt.bfloat16, tag="t1")
            t2 = rope_tmp.tile([128, 4, 5, 32], dt.bfloat16, tag="t2")
            t3 = rope_tmp.tile([128, 4, 5, 32], dt.bfloat16, tag="t3")
            t4_ = rope_tmp.tile([128, 4, 5, 32], dt.bfloat16, tag="t4")
            nc.vector.tensor_mul(t1[:], x1, cos)
            nc.vector.tensor_mul(t2[:], x2, sin)
            nc.vector.tensor_mul(t3[:], x1, sin)
            nc.vector.tensor_mul(t4_[:], x2, cos)
            nc.vector.tensor_sub(x1, t1[:], t2[:])
            nc.vector.tensor_add(x2, t3[:], t4_[:])

            kt = persist.tile([128, 4, 128], dt.bfloat16, name=f"kt{jc}")
            kt_sb.append(kt)
            for t4 in range(4):
                nc.sync.dma_start_transpose(
                    qt_sb[0][jc][:, ts(t4, 128)], qn[:, t4, 0:128]
                )
                nc.sync.dma_start_transpose(
                    qt_sb[1][jc][:, ts(t4, 128)], qn[:, t4, 128:256]
                )
                # rows 0-63 <- kT (k dims 256:320); rows 64-127 junk, fixed below
                nc.sync.dma_start_transpose(kt[:, t4, :], qn[:, t4, 256:384])
            nc.gpsimd.dma_start(kt[64:128, :, :], kt[0:64, :, :])

        # ---- phase 3+4: attention + projection per 512-wide q chunk ----
        # y^T[65, q] = [1|v]^T @ P^T over k tiles; row 0 = softmax denominator
        for j in range(NCHUNK):
            y_ps = y_ps_pool.tile([65, 4, 512], dt.float32)  # 4 psum banks
            for i in range(4 * j + 4):  # k tiles
                ic, i4 = divmod(i, 4)
                off = max(0, 128 * i - 512 * j)  # causal: valid q >= 128*i
                for h in range(4):
                    u = h % 2  # kT copy at partitions 64u..64u+63
                    s_ps = s_ps_pool.tile([128, 512], dt.float32, name="s_ps")
                    nc.tensor.matmul(
                        s_ps[:, off:512],
                        lhsT=kt_sb[ic][ts(u, 64), i4, :],
                        rhs=qt_sb[h // 2][j][ts(u, 64), off:512],
                        start=True,
                        stop=True,
                    )
                    p_t = p_pool.tile([128, 512], dt.bfloat16, name="p_t")
                    nc.scalar.activation(
                        p_t[:, off:512], s_ps[:, off:512], AF.Exp, scale=0.125
                    )
                    if 128 * i >= 512 * j:  # diagonal block: causal mask
                        nc.vector.tensor_mul(
                            p_t[:, off : off + 128],
                            p_t[:, off : off + 128],
                            mk_sb[:],
                        )
                    nc.tensor.matmul(
                        y_ps[:, h, off:512],
                        lhsT=qkvn[ic][:, i4, 320:385],
                        rhs=p_t[:, off:512],
                        start=(i == 0),
                        stop=(i == 4 * j + 3),
                    )
            # evacuate unnormalized y + den quickly to free the psum banks
            y_sb = ysb_pool.tile([65, 4, 512], dt.float32)
            nc.vector.tensor_copy(y_sb[:], y_ps[:])
            # den row -> reciprocal -> broadcast -> scale
            nc.vector.reciprocal_approx_fast(y_sb[0:1, :, :], y_sb[0:1, :, :])
            bc = bc_pool.tile([65, 4, 512], dt.float32)
            nc.gpsimd.partition_broadcast(bc[:], y_sb[0:1, :, :], channels=65)
            for h in range(4):
                yst = yst_pool.tile([65, 512], dt.bfloat16)
                nc.vector.tensor_mul(yst[:, :], y_sb[:, h, :], bc[:, h, :])
                nc.scalar.dma_start(
                    ynt[h // 2][j][ts(h % 2, 64), :], yst[1:65, :]
                )
            # projection for this chunk's 4 token tiles
            for t4 in range(4):
                tt = 4 * j + t4
                po = po_pool.tile([128, 1024], dt.float32)
                for nn2 in range(2):
                    ps = pr_ps_pool.tile([128, 512], dt.float32, tag="t", name="pr_ps")
                    for dtile in range(2):
                        nc.tensor.matmul(
                            ps[:],
                            lhsT=ynt[dtile][j][:, ts(t4, 128)],
                            rhs=wo_sb[:, dtile, ts(nn2, 512)],
                            start=(dtile == 0),
                            stop=(dtile == 1),
                        )
                    nc.vector.tensor_copy(po[:, ts(nn2, 512)], ps[:])
                nc.scalar.dma_start(
                    out_d.ap()[ts(tt, 128), :], po[:]
                )

    nc.finalize()
    return nc


def _host_inputs(x, W_qkv, W_proj):
    """Per-core input maps (host-side sharding + partition-major layout)."""
    perm = np.concatenate([np.arange(0, HD, 2), np.arange(1, HD, 2)])  # even|odd
    inv = 1.0 / THETA ** (np.arange(0, HD, 2, dtype=np.float64) / HD)  # [32]
    ang = np.arange(T, dtype=np.float64)[:, None] * inv[None, :]       # [T, 32]
    cos5 = np.tile(np.cos(ang), (1, 5))
    sin5 = np.tile(np.sin(ang), (1, 5))
    cs = np.concatenate([cos5, sin5], axis=1).astype(BF16)             # [T, 320]
    # [T, 320] -> [128, TT*320] partition-major (token t = n*128 + p)
    cs_pm = np.ascontiguousarray(
        cs.reshape(TT, 128, 320).transpose(1, 0, 2).reshape(128, TT * 320)
    )
    masku = np.triu(np.ones((128, 128), dtype=np.float32)).astype(BF16)

    def part_major(a, p=128):  # [R, cols] with R = n*p -> [p, n*cols]
        R, cols = a.shape
        n = R // p
        return np.ascontiguousarray(
            a.reshape(n, p, cols).transpose(1, 0, 2).reshape(p, n * cols)
        )

    in_maps = []
    for core in range(8):
        b, hg = divmod(core, 4)
        xT = x[b].T.astype(BF16)                                       # [C, T]
        # [C, T] -> [128, jc*(8*512)]: (c p), (jc t') -> p, jc, c, t'
        xt_pm = np.ascontiguousarray(
            xT.reshape(8, 128, NCHUNK, 512)
            .transpose(1, 2, 0, 3)
            .reshape(128, NCHUNK * 8 * 512)
        )
        cols = []
        for h in range(hg * NQ, hg * NQ + NQ):
            cols.append(W_qkv[:, h * HD : (h + 1) * HD][:, perm])
        kblk = W_qkv[:, NH * HD + hg * HD : NH * HD + (hg + 1) * HD][:, perm]
        vblk = W_qkv[:, (NH + NKV) * HD + hg * HD : (NH + NKV) * HD + (hg + 1) * HD]
        w384 = np.concatenate(cols + [kblk, vblk], axis=1).astype(BF16)
        wo = W_proj[hg * NQ * HD : (hg + 1) * NQ * HD, :].astype(BF16)
        in_maps.append(
            {
                "xT": xt_pm,
                "w384": part_major(w384),
                "wo": part_major(wo),
                "cs": cs_pm,
                "masku": masku,
            }
        )
    return in_maps


def _run(in_maps):
    from concourse.bass_utils import run_bass_kernel_spmd

    if "nc" not in _CACHE:
        _CACHE["nc"] = _build()
    return run_bass_kernel_spmd(_CACHE["nc"], in_maps, core_ids=list(range(8)))


def kernel(x, W_qkv, W_proj):
    x = np.asarray(x, dtype=np.float32)
    W_qkv = np.asarray(W_qkv, dtype=np.float32)
    W_proj = np.asarray(W_proj, dtype=np.float32)
    res = _run(_host_inputs(x, W_qkv, W_proj))
    out = np.zeros((B, T, C), dtype=np.float32)
    for core in range(8):
        b = core // 4
        out[b] += res.results[core]["out"]
    return out


# revision 15
# speedup vs baseline: 529.7340x; 1.8401x over previous
"""Causal GQA self-attention (B=2, T=2048, C=1024, 16 q-heads / 4 kv-heads,
RoPE, causal softmax, output projection) on 8 Trainium2 NeuronCores.

Sharding: core c = b*4 + hg handles batch b (2-way data parallel) and
head-group hg (4-way tensor parallel: its 4 q-heads + their shared kv head).
W_qkv is column-sharded, W_proj row-sharded; each core emits a partial
projection [2048, 1024] and the host sums the 4 partials per batch.

Device pipeline per core (bf16 matmul inputs, fp32 PSUM accumulate):
  1. qkv = x @ W_qkv_shard, natural layout [token, 320|1|64] (q,k | ones | v)
  2. RoPE on q,k in natural layout (head-dim pairs pre-permuted even|odd on
     the host, so the rotation is two contiguous 32-blocks per head)
  3. DMA-xbar-transpose q,k -> qT [256, T]; kT duplicated to partitions 0-63
     and 64-127 (row-packed score matmuls, 2 heads concurrent in the array)
  4. flash-style per 512-wide q chunk: S^T = k @ qT (K=64), exp on ScalarE
     (1/sqrt(64) folded into the activation scale), causal masking only on
     diagonal 128-blocks, y^T[1+64, q] += [1|v]^T @ P^T (v_aug stationary)
  5. y^T row 0 is the softmax denominator: reciprocal_approx + GPSIMD
     partition_broadcast + one fused scale-evacuate multiply
  6. out = yT.T @ W_proj_shard
"""

import sys

if "/opt/trn_rl_repo" not in sys.path:
    sys.path.insert(0, "/opt/trn_rl_repo")

import numpy as np
import ml_dtypes

B, T, C = 2, 2048, 1024
NH, NKV, HD = 16, 4, 64
THETA = 10000.0
NQ = NH // NKV          # q heads per core = 4
TT = T // 128           # 16 token tiles
NCHUNK = T // 512       # 4 q-chunks
BF16 = ml_dtypes.bfloat16

_CACHE = {}


def _build():
    """Build the SPMD Bass program (identical on all 8 cores)."""
    import concourse.mybir as mybir
    import concourse.tile as tile
    from concourse import bacc
    from concourse.bass import ts
    from contextlib import ExitStack

    dt = mybir.dt
    AF = mybir.ActivationFunctionType

    nc = bacc.Bacc("TRN2", target_bir_lowering=False, debug=False, num_devices=8)

    # host pre-shuffled, partition-major inputs (contiguous per partition)
    xt_d = nc.declare_dram_parameter("xT", [128, 8 * T], dt.bfloat16, isOutput=False)
    w_d = nc.declare_dram_parameter("w384", [128, 8 * 384], dt.bfloat16, isOutput=False)
    wo_d = nc.declare_dram_parameter("wo", [128, 2 * C], dt.bfloat16, isOutput=False)
    cs_d = nc.declare_dram_parameter("cs", [128, TT * 320], dt.bfloat16, isOutput=False)
    mk_d = nc.declare_dram_parameter("masku", [128, 128], dt.bfloat16, isOutput=False)
    out_d = nc.declare_dram_parameter("out", [T, C], dt.bfloat16, isOutput=True)

    with tile.TileContext(nc) as tc, ExitStack() as ctx:
        persist = ctx.enter_context(tc.tile_pool(name="persist", bufs=1))
        rope_tmp = ctx.enter_context(tc.tile_pool(name="rope_tmp", bufs=4))
        p_pool = ctx.enter_context(tc.tile_pool(name="p_pool", bufs=8))
        po_pool = ctx.enter_context(tc.tile_pool(name="po", bufs=4))
        ysb_pool = ctx.enter_context(tc.tile_pool(name="ysb", bufs=2))
        bc_pool = ctx.enter_context(tc.tile_pool(name="bc", bufs=2))
        yst_pool = ctx.enter_context(tc.tile_pool(name="yst", bufs=4))
        s_ps_pool = ctx.enter_context(tc.tile_pool(name="s_ps", bufs=3, space="PSUM"))
        pr_ps_pool = ctx.enter_context(tc.tile_pool(name="pr_ps", bufs=1, space="PSUM"))
        y_ps_pool = ctx.enter_context(tc.tile_pool(name="y_ps", bufs=1, space="PSUM"))

        # ---- persistent SBUF (split tiles for fine-grained scheduling) ----
        w_sb = persist.tile([128, 8, 384], dt.bfloat16)
        nc.sync.dma_start(w_sb[:], w_d.ap().rearrange("p (c n) -> p c n", c=8))
        xt_sb = []
        cs_sb = []
        for jc in range(NCHUNK):
            xt = persist.tile([128, 8, 512], dt.bfloat16, name=f"xtc{jc}")
            nc.sync.dma_start(
                xt[:],
                xt_d.ap()[:, ts(jc, 8 * 512)].rearrange("p (c t) -> p c t", c=8),
            )
            xt_sb.append(xt)
            cst = persist.tile([128, 4, 320], dt.bfloat16, name=f"cs{jc}")
            nc.sync.dma_start(
                cst[:],
                cs_d.ap()[:, ts(jc, 4 * 320)].rearrange("p (n d) -> p n d", n=4),
            )
            cs_sb.append(cst)
            if jc == 0:
                mk_sb = persist.tile([128, 128], dt.bfloat16)
                nc.sync.dma_start(mk_sb[:], mk_d.ap())
        wo_sb = persist.tile([128, 2, C], dt.bfloat16)
        nc.sync.dma_start(wo_sb[:], wo_d.ap().rearrange("p (c n) -> p c n", c=2))

        qkvn = []   # per chunk: [128, 4, 385] = q,k rot | ones | v
        qt_sb = [[None] * NCHUNK for _ in range(2)]   # [dimtile][chunk] [128,512]
        kt_sb = []  # per chunk [128, 4, 128]: kT at rows 0-63 and 64-127
        ynt = [[None] * NCHUNK for _ in range(2)]     # [dimtile][chunk] [128,512]
        for d in range(2):
            for j in range(NCHUNK):
                qt_sb[d][j] = persist.tile([128, 512], dt.bfloat16, name=f"qt{d}_{j}")
                ynt[d][j] = persist.tile([128, 512], dt.bfloat16, name=f"ynt{d}_{j}")

        # ---- phase 1+2: qkv, rope, transposes (per 4-token-tile chunk) ----
        for jc in range(NCHUNK):
            qn = persist.tile([128, 4, 385], dt.bfloat16, name=f"qkvn{jc}")
            qkvn.append(qn)
            for t4 in range(4):
                tt = 4 * jc + t4
                ps = pr_ps_pool.tile([128, 384], dt.float32, tag="t", name="qkv_ps")
                for c in range(8):
                    nc.tensor.matmul(
                        ps[:],
                        lhsT=xt_sb[jc][:, c, ts(t4, 128)],
                        rhs=w_sb[:, c, :],
                        start=(c == 0),
                        stop=(c == 7),
                    )
                nc.vector.tensor_copy(qn[:, t4, 0:320], ps[:, 0:320])
                nc.vector.tensor_copy(qn[:, t4, 321:385], ps[:, 320:384])
            nc.gpsimd.memset(qn[:, :, 320:321], 1.0)

            base = qn[:, :, 0:320].rearrange("p f (h d) -> p f h d", h=5)
            x1 = base[:, :, :, 0:32]
            x2 = base[:, :, :, 32:64]
            csc = cs_sb[jc][:]
            cos = csc[:, :, 0:160].rearrange("p f (h d) -> p f h d", h=5)
            sin = csc[:, :, 160:320].rearrange("p f (h d) -> p f h d", h=5)
            t1 = rope_tmp.tile([128, 4, 5, 32], dt.bfloat16, tag="t1")
            t2 = rope_tmp.tile([128, 4, 5, 32], dt.bfloat16, tag="t2")
            t3 = rope_tmp.tile([128, 4, 5, 32], dt.bfloat16, tag="t3")
            t4_ = rope_tmp.tile([128, 4, 5, 32], dt.bfloat16, tag="t4")
            nc.vector.tensor_mul(t1[:], x1, cos)
            nc.vector.tensor_mul(t2[:], x2, sin)
            nc.vector.tensor_mul(t3[:], x1, sin)
            nc.vector.tensor_mul(t4_[:], x2, cos)
            nc.vector.tensor_sub(x1, t1[:], t2[:])
            nc.vector.tensor_add(x2, t3[:], t4_[:])

            kt = persist.tile([128, 4, 128], dt.bfloat16, name=f"kt{jc}")
            kt_sb.append(kt)
            for t4 in range(4):
                nc.sync.dma_start_transpose(
                    qt_sb[0][jc][:, ts(t4, 128)], qn[:, t4, 0:128]
                )
                nc.sync.dma_start_transpose(
                    qt_sb[1][jc][:, ts(t4, 128)], qn[:, t4, 128:256]
                )
                # rows 0-63 <- kT (k dims 256:320); rows 64-127 junk, fixed below
                nc.sync.dma_start_transpose(kt[:, t4, :], qn[:, t4, 256:384])
            nc.gpsimd.dma_start(kt[64:128, :, :], kt[0:64, :, :])

        # ---- phase 3+4: attention + projection per 512-wide q chunk ----
        # y^T[65, q] = [1|v]^T @ P^T over k tiles; row 0 = softmax denominator
        for j in range(NCHUNK):
            y_ps = y_ps_pool.tile([65, 4, 512], dt.float32)  # 4 psum banks
            for i in range(4 * j + 4):  # k tiles
                ic, i4 = divmod(i, 4)
                off = max(0, 128 * i - 512 * j)  # causal: valid q >= 128*i
                for h in range(4):
                    u = h % 2  # kT copy at partitions 64u..64u+63
                    s_ps = s_ps_pool.tile([128, 512], dt.float32, tag="s", name="s_ps")
                    nc.tensor.matmul(
                        s_ps[:, off:512],
                        lhsT=kt_sb[ic][ts(u, 64), i4, :],
                        rhs=qt_sb[h // 2][j][ts(u, 64), off:512],
                        start=True,
                        stop=True,
                    )
                    p_t = p_pool.tile([128, 512], dt.bfloat16, name="p_t")
                    nc.scalar.activation(
                        p_t[:, off:512], s_ps[:, off:512], AF.Exp, scale=0.125
                    )
                    if 128 * i >= 512 * j:  # diagonal block: causal mask
                        # keep where q_local - k_local >= 0, else 0
                        nc.gpsimd.affine_select(
                            p_t[:, off : off + 128],
                            p_t[:, off : off + 128],
                            pattern=[[1, 128]],
                            compare_op=mybir.AluOpType.is_ge,
                            fill=0.0,
                            base=0,
                            channel_multiplier=-1,
                        )
                    nc.tensor.matmul(
                        y_ps[:, h, off:512],
                        lhsT=qkvn[ic][:, i4, 320:385],
                        rhs=p_t[:, off:512],
                        start=(i == 0),
                        stop=(i == 4 * j + 3),
                    )
            # evacuate unnormalized y + den quickly to free the psum banks
            y_sb = ysb_pool.tile([65, 4, 512], dt.float32)
            nc.vector.tensor_copy(y_sb[:], y_ps[:])
            # den row -> reciprocal -> broadcast -> scale
            nc.vector.reciprocal_approx_fast(y_sb[0:1, :, :], y_sb[0:1, :, :])
            bc = bc_pool.tile([65, 4, 512], dt.float32)
            nc.gpsimd.partition_broadcast(bc[:], y_sb[0:1, :, :], channels=65)
            for h in range(4):
                yst = yst_pool.tile([65, 512], dt.bfloat16)
                nc.vector.tensor_mul(yst[:, :], y_sb[:, h, :], bc[:, h, :])
                nc.scalar.dma_start(
                    ynt[h // 2][j][ts(h % 2, 64), :], yst[1:65, :]
                )
            # projection for this chunk's 4 token tiles
            for t4 in range(4):
                tt = 4 * j + t4
                po = po_pool.tile([128, 1024], dt.bfloat16)
                for nn2 in range(2):
                    if j == NCHUNK - 1:
                        ps = s_ps_pool.tile(
                            [128, 512], dt.float32, tag="s", name="pr3s_ps"
                        )
                    else:
                        ps = pr_ps_pool.tile(
                            [128, 512], dt.float32, tag="t", name="pr_ps"
                        )
                    for dtile in range(2):
                        nc.tensor.matmul(
                            ps[:],
                            lhsT=ynt[dtile][j][:, ts(t4, 128)],
                            rhs=wo_sb[:, dtile, ts(nn2, 512)],
                            start=(dtile == 0),
                            stop=(dtile == 1),
                        )
                    if j == NCHUNK - 1:
                        nc.scalar.copy(po[:, ts(nn2, 512)], ps[:])
                    else:
                        nc.vector.tensor_copy(po[:, ts(nn2, 512)], ps[:])
                nc.scalar.dma_start(
                    out_d.ap()[ts(tt, 128), :], po[:]
                )

    nc.finalize()
    return nc


def _host_inputs(x, W_qkv, W_proj):
    """Per-core input maps (host-side sharding + partition-major layout)."""
    perm = np.concatenate([np.arange(0, HD, 2), np.arange(1, HD, 2)])  # even|odd
    inv = 1.0 / THETA ** (np.arange(0, HD, 2, dtype=np.float64) / HD)  # [32]
    ang = np.arange(T, dtype=np.float64)[:, None] * inv[None, :]       # [T, 32]
    cos5 = np.tile(np.cos(ang), (1, 5))
    sin5 = np.tile(np.sin(ang), (1, 5))
    cs = np.concatenate([cos5, sin5], axis=1).astype(BF16)             # [T, 320]
    # [T, 320] -> [128, TT*320] partition-major (token t = n*128 + p)
    cs_pm = np.ascontiguousarray(
        cs.reshape(TT, 128, 320).transpose(1, 0, 2).reshape(128, TT * 320)
    )
    masku = np.triu(np.ones((128, 128), dtype=np.float32)).astype(BF16)

    def part_major(a, p=128):  # [R, cols] with R = n*p -> [p, n*cols]
        R, cols = a.shape
        n = R // p
        return np.ascontiguousarray(
            a.reshape(n, p, cols).transpose(1, 0, 2).reshape(p, n * cols)
        )

    in_maps = []
    for core in range(8):
        b, hg = divmod(core, 4)
        xT = x[b].T.astype(BF16)                                       # [C, T]
        # [C, T] -> [128, jc*(8*512)]: (c p), (jc t') -> p, jc, c, t'
        xt_pm = np.ascontiguousarray(
            xT.reshape(8, 128, NCHUNK, 512)
            .transpose(1, 2, 0, 3)
            .reshape(128, NCHUNK * 8 * 512)
        )
        cols = []
        for h in range(hg * NQ, hg * NQ + NQ):
            cols.append(W_qkv[:, h * HD : (h + 1) * HD][:, perm])
        kblk = W_qkv[:, NH * HD + hg * HD : NH * HD + (hg + 1) * HD][:, perm]
        vblk = W_qkv[:, (NH + NKV) * HD + hg * HD : (NH + NKV) * HD + (hg + 1) * HD]
        w384 = np.concatenate(cols + [kblk, vblk], axis=1).astype(BF16)
        wo = W_proj[hg * NQ * HD : (hg + 1) * NQ * HD, :].astype(BF16)
        in_maps.append(
            {
                "xT": xt_pm,
                "w384": part_major(w384),
                "wo": part_major(wo),
                "cs": cs_pm,
                "masku": masku,
            }
        )
    return in_maps


def _run(in_maps):
    from concourse.bass_utils import run_bass_kernel_spmd

    if "nc" not in _CACHE:
        _CACHE["nc"] = _build()
    return run_bass_kernel_spmd(_CACHE["nc"], in_maps, core_ids=list(range(8)))


def kernel(x, W_qkv, W_proj):
    x = np.asarray(x, dtype=np.float32)
    W_qkv = np.asarray(W_qkv, dtype=np.float32)
    W_proj = np.asarray(W_proj, dtype=np.float32)
    res = _run(_host_inputs(x, W_qkv, W_proj))
    out = np.zeros((B, T, C), dtype=np.float32)
    for core in range(8):
        b = core // 4
        out[b] += res.results[core]["out"].astype(np.float32)
    return out


# revision 18
# speedup vs baseline: 632.4558x; 1.1939x over previous
"""Causal GQA self-attention (B=2, T=2048, C=1024, 16 q-heads / 4 kv-heads,
RoPE, causal softmax, output projection) on 8 Trainium2 NeuronCores.

Sharding: core c = b*4 + hg handles batch b (2-way data parallel) and
head-group hg (4-way tensor parallel: its 4 q-heads + their shared kv head).
W_qkv is column-sharded, W_proj row-sharded; each core emits a partial
projection [2048, 1024] and the host sums the 4 partials per batch.

Device pipeline per core (bf16 matmul inputs, fp32 PSUM accumulate):
  1. qkv = x @ W_qkv_shard, natural layout [token, 320|1|64] (q,k | ones | v)
  2. RoPE on q,k in natural layout (head-dim pairs pre-permuted even|odd on
     the host, so the rotation is two contiguous 32-blocks per head)
  3. DMA-xbar-transpose q,k -> qT [256, T]; kT duplicated to partitions 0-63
     and 64-127 (row-packed score matmuls, 2 heads concurrent in the array)
  4. flash-style per 512-wide q chunk: S^T = k @ qT (K=64), exp on ScalarE
     (1/sqrt(64) folded into the activation scale), causal masking only on
     diagonal 128-blocks, y^T[1+64, q] += [1|v]^T @ P^T (v_aug stationary)
  5. y^T row 0 is the softmax denominator: reciprocal_approx + GPSIMD
     partition_broadcast + one fused scale-evacuate multiply
  6. out = yT.T @ W_proj_shard
"""

import sys

if "/opt/trn_rl_repo" not in sys.path:
    sys.path.insert(0, "/opt/trn_rl_repo")

import numpy as np
import ml_dtypes

B, T, C = 2, 2048, 1024
NH, NKV, HD = 16, 4, 64
THETA = 10000.0
NQ = NH // NKV          # q heads per core = 4
TT = T // 128           # 16 token tiles
NCHUNK = T // 512       # 4 q-chunks
BF16 = ml_dtypes.bfloat16

_CACHE = {}


def _build():
    """Build the SPMD Bass program (identical on all 8 cores)."""
    import concourse.mybir as mybir
    import concourse.tile as tile
    from concourse import bacc
    from concourse.bass import ts
    from contextlib import ExitStack

    dt = mybir.dt
    AF = mybir.ActivationFunctionType

    nc = bacc.Bacc("TRN2", target_bir_lowering=False, debug=False, num_devices=8)

    # host pre-shuffled, partition-major inputs (contiguous per partition)
    xt_d = nc.declare_dram_parameter("xT", [128, 8 * T], dt.bfloat16, isOutput=False)
    w_d = nc.declare_dram_parameter("w384", [128, 8 * 384], dt.bfloat16, isOutput=False)
    wo_d = nc.declare_dram_parameter("wo", [128, 2 * C], dt.bfloat16, isOutput=False)
    cs_d = nc.declare_dram_parameter("cs", [128, TT * 320], dt.bfloat16, isOutput=False)
    mk_d = nc.declare_dram_parameter("masku", [128, 128], dt.bfloat16, isOutput=False)
    out_d = nc.declare_dram_parameter("out", [T, C], dt.bfloat16, isOutput=True)

    with tile.TileContext(nc) as tc, ExitStack() as ctx:
        persist = ctx.enter_context(tc.tile_pool(name="persist", bufs=1))
        rope_tmp = ctx.enter_context(tc.tile_pool(name="rope_tmp", bufs=4))
        p_pool = ctx.enter_context(tc.tile_pool(name="p_pool", bufs=14))
        po_pool = ctx.enter_context(tc.tile_pool(name="po", bufs=6))
        ysb_pool = ctx.enter_context(tc.tile_pool(name="ysb", bufs=3))
        bc_pool = ctx.enter_context(tc.tile_pool(name="bc", bufs=2))
        yst_pool = ctx.enter_context(tc.tile_pool(name="yst", bufs=6))
        s_ps_pool = ctx.enter_context(tc.tile_pool(name="s_ps", bufs=3, space="PSUM"))
        pr_ps_pool = ctx.enter_context(tc.tile_pool(name="pr_ps", bufs=1, space="PSUM"))
        y_ps_pool = ctx.enter_context(tc.tile_pool(name="y_ps", bufs=1, space="PSUM"))

        # ---- persistent SBUF (split tiles for fine-grained scheduling) ----
        w_sb = persist.tile([128, 8, 384], dt.bfloat16)
        nc.sync.dma_start(w_sb[:], w_d.ap().rearrange("p (c n) -> p c n", c=8))
        xt_sb = []
        cs_sb = []
        for jc in range(NCHUNK):
            xt = persist.tile([128, 8, 512], dt.bfloat16, name=f"xtc{jc}")
            nc.sync.dma_start(
                xt[:],
                xt_d.ap()[:, ts(jc, 8 * 512)].rearrange("p (c t) -> p c t", c=8),
            )
            xt_sb.append(xt)
            cst = persist.tile([128, 4, 320], dt.bfloat16, name=f"cs{jc}")
            nc.sync.dma_start(
                cst[:],
                cs_d.ap()[:, ts(jc, 4 * 320)].rearrange("p (n d) -> p n d", n=4),
            )
            cs_sb.append(cst)
            if jc == 0:
                mk_sb = persist.tile([128, 128], dt.bfloat16)
                nc.sync.dma_start(mk_sb[:], mk_d.ap())
        wo_sb = persist.tile([128, 2, C], dt.bfloat16)
        nc.sync.dma_start(wo_sb[:], wo_d.ap().rearrange("p (c n) -> p c n", c=2))

        qkvn = []   # per chunk: [128, 4, 385] = q,k rot | ones | v
        qt_sb = [[None] * NCHUNK for _ in range(2)]   # [dimtile][chunk] [128,512]
        kt_sb = []  # per chunk: (ktA rows 0-63, ktB rows 64-127) [128, 4, 128]
        ynt = [[None] * NCHUNK for _ in range(2)]     # [dimtile][chunk] [128,512]
        for d in range(2):
            for j in range(NCHUNK):
                qt_sb[d][j] = persist.tile([128, 512], dt.bfloat16, name=f"qt{d}_{j}")
                ynt[d][j] = persist.tile([128, 512], dt.bfloat16, name=f"ynt{d}_{j}")

        # ---- phase 1+2: qkv, rope, transposes (per 4-token-tile chunk) ----
        for jc in range(NCHUNK):
            qn = persist.tile([128, 4, 385], dt.bfloat16, name=f"qkvn{jc}")
            qkvn.append(qn)
            for t4 in range(4):
                tt = 4 * jc + t4
                ps = pr_ps_pool.tile([128, 384], dt.float32, tag="t", name="qkv_ps")
                for c in range(8):
                    nc.tensor.matmul(
                        ps[:],
                        lhsT=xt_sb[jc][:, c, ts(t4, 128)],
                        rhs=w_sb[:, c, :],
                        start=(c == 0),
                        stop=(c == 7),
                    )
                nc.vector.tensor_copy(qn[:, t4, 0:320], ps[:, 0:320])
                nc.vector.tensor_copy(qn[:, t4, 321:385], ps[:, 320:384])
            nc.gpsimd.memset(qn[:, :, 320:321], 1.0)

            base = qn[:, :, 0:320].rearrange("p f (h d) -> p f h d", h=5)
            x1 = base[:, :, :, 0:32]
            x2 = base[:, :, :, 32:64]
            csc = cs_sb[jc][:]
            cos = csc[:, :, 0:160].rearrange("p f (h d) -> p f h d", h=5)
            sin = csc[:, :, 160:320].rearrange("p f (h d) -> p f h d", h=5)
            t1 = rope_tmp.tile([128, 4, 5, 32], dt.bfloat16, tag="t1")
            t2 = rope_tmp.tile([128, 4, 5, 32], dt.bfloat16, tag="t2")
            t3 = rope_tmp.tile([128, 4, 5, 32], dt.bfloat16, tag="t3")
            t4_ = rope_tmp.tile([128, 4, 5, 32], dt.bfloat16, tag="t4")
            nc.vector.tensor_mul(t1[:], x1, cos)
            nc.vector.tensor_mul(t2[:], x2, sin)
            nc.vector.tensor_mul(t3[:], x1, sin)
            nc.vector.tensor_mul(t4_[:], x2, cos)
            nc.vector.tensor_sub(x1, t1[:], t2[:])
            nc.vector.tensor_add(x2, t3[:], t4_[:])

            kta = persist.tile([128, 4, 128], dt.bfloat16, name=f"kta{jc}")
            ktb = persist.tile([128, 4, 128], dt.bfloat16, name=f"ktb{jc}")
            kt_sb.append((kta, ktb))
            for t4 in range(4):
                nc.sync.dma_start_transpose(
                    qt_sb[0][jc][:, ts(t4, 128)], qn[:, t4, 0:128]
                )
                nc.sync.dma_start_transpose(
                    qt_sb[1][jc][:, ts(t4, 128)], qn[:, t4, 128:256]
                )
                # rows 0-63 <- kT (k dims 256:320); rows 64-127 junk (v cols)
                nc.sync.dma_start_transpose(kta[:, t4, :], qn[:, t4, 256:384])
            # duplicate kT into rows 64-127 of a separate tile so u=0 score
            # matmuls don't wait on this copy (whole-tile dependency)
            nc.gpsimd.dma_start(ktb[64:128, :, :], kta[0:64, :, :])

        # ---- phase 3+4: attention + projection per 512-wide q chunk ----
        # y^T[65, q] = [1|v]^T @ P^T over k tiles; row 0 = softmax denominator
        for j in range(NCHUNK):
            y_ps = y_ps_pool.tile([65, 4, 512], dt.float32)  # 4 psum banks
            for i in range(4 * j + 4):  # k tiles
                ic, i4 = divmod(i, 4)
                off = max(0, 128 * i - 512 * j)  # causal: valid q >= 128*i
                for h in range(4):
                    u = h % 2  # kT copy at partitions 64u..64u+63
                    s_ps = s_ps_pool.tile([128, 512], dt.float32, tag="s", name="s_ps")
                    nc.tensor.matmul(
                        s_ps[:, off:512],
                        lhsT=kt_sb[ic][u][ts(u, 64), i4, :],
                        rhs=qt_sb[h // 2][j][ts(u, 64), off:512],
                        start=True,
                        stop=True,
                    )
                    p_t = p_pool.tile([128, 512], dt.bfloat16, name="p_t")
                    nc.scalar.activation(
                        p_t[:, off:512], s_ps[:, off:512], AF.Exp, scale=0.125
                    )
                    if 128 * i >= 512 * j:  # diagonal block: causal mask
                        # keep where q_local - k_local >= 0, else 0
                        nc.gpsimd.affine_select(
                            p_t[:, off : off + 128],
                            p_t[:, off : off + 128],
                            pattern=[[1, 128]],
                            compare_op=mybir.AluOpType.is_ge,
                            fill=0.0,
                            base=0,
                            channel_multiplier=-1,
                        )
                    nc.tensor.matmul(
                        y_ps[:, h, off:512],
                        lhsT=qkvn[ic][:, i4, 320:385],
                        rhs=p_t[:, off:512],
                        start=(i == 0),
                        stop=(i == 4 * j + 3),
                    )
            # evacuate unnormalized y + den quickly to free the psum banks
            y_sb = ysb_pool.tile([65, 4, 512], dt.float32)
            nc.vector.tensor_copy(y_sb[:], y_ps[:])
            # den row -> reciprocal -> broadcast -> scale
            nc.vector.reciprocal_approx_fast(y_sb[0:1, :, :], y_sb[0:1, :, :])
            bc = bc_pool.tile([65, 4, 512], dt.float32)
            nc.gpsimd.partition_broadcast(bc[:], y_sb[0:1, :, :], channels=65)
            for h in range(4):
                yst = yst_pool.tile([65, 512], dt.bfloat16)
                nc.vector.tensor_mul(yst[:, :], y_sb[:, h, :], bc[:, h, :])
                nc.scalar.dma_start(
                    ynt[h // 2][j][ts(h % 2, 64), :], yst[1:65, :]
                )
            # projection for this chunk's 4 token tiles
            for t4 in range(4):
                tt = 4 * j + t4
                po = po_pool.tile([128, 1024], dt.bfloat16)
                for nn2 in range(2):
                    if j == NCHUNK - 1:
                        ps = s_ps_pool.tile(
                            [128, 512], dt.float32, tag="s", name="pr3s_ps"
                        )
                    else:
                        ps = pr_ps_pool.tile(
                            [128, 512], dt.float32, tag="t", name="pr_ps"
                        )
                    for dtile in range(2):
                        nc.tensor.matmul(
                            ps[:],
                            lhsT=ynt[dtile][j][:, ts(t4, 128)],
                            rhs=wo_sb[:, dtile, ts(nn2, 512)],
                            start=(dtile == 0),
                            stop=(dtile == 1),
                        )
                    if j == NCHUNK - 1:
                        nc.scalar.copy(po[:, ts(nn2, 512)], ps[:])
                    else:
                        nc.vector.tensor_copy(po[:, ts(nn2, 512)], ps[:])
                nc.scalar.dma_start(
                    out_d.ap()[ts(tt, 128), :], po[:]
                )

    nc.finalize()
    return nc


def _host_inputs(x, W_qkv, W_proj):
    """Per-core input maps (host-side sharding + partition-major layout)."""
    perm = np.concatenate([np.arange(0, HD, 2), np.arange(1, HD, 2)])  # even|odd
    inv = 1.0 / THETA ** (np.arange(0, HD, 2, dtype=np.float64) / HD)  # [32]
    ang = np.arange(T, dtype=np.float64)[:, None] * inv[None, :]       # [T, 32]
    cos5 = np.tile(np.cos(ang), (1, 5))
    sin5 = np.tile(np.sin(ang), (1, 5))
    cs = np.concatenate([cos5, sin5], axis=1).astype(BF16)             # [T, 320]
    # [T, 320] -> [128, TT*320] partition-major (token t = n*128 + p)
    cs_pm = np.ascontiguousarray(
        cs.reshape(TT, 128, 320).transpose(1, 0, 2).reshape(128, TT * 320)
    )
    masku = np.triu(np.ones((128, 128), dtype=np.float32)).astype(BF16)

    def part_major(a, p=128):  # [R, cols] with R = n*p -> [p, n*cols]
        R, cols = a.shape
        n = R // p
        return np.ascontiguousarray(
            a.reshape(n, p, cols).transpose(1, 0, 2).reshape(p, n * cols)
        )

    in_maps = []
    for core in range(8):
        b, hg = divmod(core, 4)
        xT = x[b].T.astype(BF16)                                       # [C, T]
        # [C, T] -> [128, jc*(8*512)]: (c p), (jc t') -> p, jc, c, t'
        xt_pm = np.ascontiguousarray(
            xT.reshape(8, 128, NCHUNK, 512)
            .transpose(1, 2, 0, 3)
            .reshape(128, NCHUNK * 8 * 512)
        )
        cols = []
        for h in range(hg * NQ, hg * NQ + NQ):
            cols.append(W_qkv[:, h * HD : (h + 1) * HD][:, perm])
        kblk = W_qkv[:, NH * HD + hg * HD : NH * HD + (hg + 1) * HD][:, perm]
        vblk = W_qkv[:, (NH + NKV) * HD + hg * HD : (NH + NKV) * HD + (hg + 1) * HD]
        w384 = np.concatenate(cols + [kblk, vblk], axis=1).astype(BF16)
        wo = W_proj[hg * NQ * HD : (hg + 1) * NQ * HD, :].astype(BF16)
        in_maps.append(
            {
                "xT": xt_pm,
                "w384": part_major(w384),
                "wo": part_major(wo),
                "cs": cs_pm,
                "masku": masku,
            }
        )
    return in_maps


def _run(in_maps):
    from concourse.bass_utils import run_bass_kernel_spmd

    if "nc" not in _CACHE:
        _CACHE["nc"] = _build()
    return run_bass_kernel_spmd(_CACHE["nc"], in_maps, core_ids=list(range(8)))


def kernel(x, W_qkv, W_proj):
    x = np.asarray(x, dtype=np.float32)
    W_qkv = np.asarray(W_qkv, dtype=np.float32)
    W_proj = np.asarray(W_proj, dtype=np.float32)
    res = _run(_host_inputs(x, W_qkv, W_proj))
    out = np.zeros((B, T, C), dtype=np.float32)
    for core in range(8):
        b = core // 4
        out[b] += res.results[core]["out"].astype(np.float32)
    return out


# revision 19
# speedup vs baseline: 636.8710x; 1.0070x over previous
"""Causal GQA self-attention (B=2, T=2048, C=1024, 16 q-heads / 4 kv-heads,
RoPE, causal softmax, output projection) on 8 Trainium2 NeuronCores.

Sharding: core c = b*4 + hg handles batch b (2-way data parallel) and
head-group hg (4-way tensor parallel: its 4 q-heads + their shared kv head).
W_qkv is column-sharded, W_proj row-sharded; each core emits a partial
projection [2048, 1024] and the host sums the 4 partials per batch.

Device pipeline per core (bf16 matmul inputs, fp32 PSUM accumulate):
  1. qkv = x @ W_qkv_shard, natural layout [token, 320|1|64] (q,k | ones | v)
  2. RoPE on q,k in natural layout (head-dim pairs pre-permuted even|odd on
     the host, so the rotation is two contiguous 32-blocks per head)
  3. DMA-xbar-transpose q,k -> qT [256, T]; kT duplicated to partitions 0-63
     and 64-127 (row-packed score matmuls, 2 heads concurrent in the array)
  4. flash-style per 512-wide q chunk: S^T = k @ qT (K=64), exp on ScalarE
     (1/sqrt(64) folded into the activation scale), causal masking only on
     diagonal 128-blocks, y^T[1+64, q] += [1|v]^T @ P^T (v_aug stationary)
  5. y^T row 0 is the softmax denominator: reciprocal_approx + GPSIMD
     partition_broadcast + one fused scale-evacuate multiply
  6. out = yT.T @ W_proj_shard
"""

import sys

if "/opt/trn_rl_repo" not in sys.path:
    sys.path.insert(0, "/opt/trn_rl_repo")

import numpy as np
import ml_dtypes

B, T, C = 2, 2048, 1024
NH, NKV, HD = 16, 4, 64
THETA = 10000.0
NQ = NH // NKV          # q heads per core = 4
TT = T // 128           # 16 token tiles
NCHUNK = T // 512       # 4 q-chunks
BF16 = ml_dtypes.bfloat16

_CACHE = {}


def _build():
    """Build the SPMD Bass program (identical on all 8 cores)."""
    import concourse.mybir as mybir
    import concourse.tile as tile
    from concourse import bacc
    from concourse.bass import ts
    from contextlib import ExitStack

    dt = mybir.dt
    AF = mybir.ActivationFunctionType

    nc = bacc.Bacc("TRN2", target_bir_lowering=False, debug=False, num_devices=8)

    # host pre-shuffled, partition-major inputs (contiguous per partition)
    xt_d = nc.declare_dram_parameter("xT", [128, 8 * T], dt.bfloat16, isOutput=False)
    w_d = nc.declare_dram_parameter("w384", [128, 8 * 384], dt.bfloat16, isOutput=False)
    wo_d = nc.declare_dram_parameter("wo", [128, 2 * C], dt.bfloat16, isOutput=False)
    cs_d = nc.declare_dram_parameter("cs", [128, TT * 320], dt.bfloat16, isOutput=False)
    mk_d = nc.declare_dram_parameter("masku", [128, 128], dt.bfloat16, isOutput=False)
    out_d = nc.declare_dram_parameter("out", [T, C], dt.bfloat16, isOutput=True)

    with tile.TileContext(nc) as tc, ExitStack() as ctx:
        persist = ctx.enter_context(tc.tile_pool(name="persist", bufs=1))
        rope_tmp = ctx.enter_context(tc.tile_pool(name="rope_tmp", bufs=4))
        p_pool = ctx.enter_context(tc.tile_pool(name="p_pool", bufs=14))
        po_pool = ctx.enter_context(tc.tile_pool(name="po", bufs=6))
        ysb_pool = ctx.enter_context(tc.tile_pool(name="ysb", bufs=3))
        bc_pool = ctx.enter_context(tc.tile_pool(name="bc", bufs=2))
        yst_pool = ctx.enter_context(tc.tile_pool(name="yst", bufs=6))
        s_ps_pool = ctx.enter_context(tc.tile_pool(name="s_ps", bufs=3, space="PSUM"))
        pr_ps_pool = ctx.enter_context(tc.tile_pool(name="pr_ps", bufs=1, space="PSUM"))
        y_ps_pool = ctx.enter_context(tc.tile_pool(name="y_ps", bufs=1, space="PSUM"))

        # ---- persistent SBUF (split tiles for fine-grained scheduling) ----
        w_sb = persist.tile([128, 8, 384], dt.bfloat16)
        nc.sync.dma_start(w_sb[:], w_d.ap().rearrange("p (c n) -> p c n", c=8))
        xt_sb = []
        cs_sb = []
        for jc in range(NCHUNK):
            xt = persist.tile([128, 8, 512], dt.bfloat16, name=f"xtc{jc}")
            nc.sync.dma_start(
                xt[:],
                xt_d.ap()[:, ts(jc, 8 * 512)].rearrange("p (c t) -> p c t", c=8),
            )
            xt_sb.append(xt)
            cst = persist.tile([128, 4, 320], dt.bfloat16, name=f"cs{jc}")
            nc.sync.dma_start(
                cst[:],
                cs_d.ap()[:, ts(jc, 4 * 320)].rearrange("p (n d) -> p n d", n=4),
            )
            cs_sb.append(cst)
            if jc == 0:
                mk_sb = persist.tile([128, 128], dt.bfloat16)
                nc.sync.dma_start(mk_sb[:], mk_d.ap())
        wo_sb = persist.tile([128, 2, C], dt.bfloat16)
        nc.sync.dma_start(wo_sb[:], wo_d.ap().rearrange("p (c n) -> p c n", c=2))

        qkvn = []   # per chunk: [128, 4, 385] = q,k rot | ones | v
        qt_sb = [[None] * NCHUNK for _ in range(2)]   # [dimtile][chunk] [128,512]
        kt_sb = []  # per chunk: (ktA rows 0-63, ktB rows 64-127) [128, 4, 128]
        ynt = [[None] * NCHUNK for _ in range(2)]     # [dimtile][chunk] [128,512]
        for d in range(2):
            for j in range(NCHUNK):
                qt_sb[d][j] = persist.tile([128, 512], dt.bfloat16, name=f"qt{d}_{j}")
                ynt[d][j] = persist.tile([128, 512], dt.bfloat16, name=f"ynt{d}_{j}")

        # ---- phase 1+2: qkv, rope, transposes (per 4-token-tile chunk) ----
        for jc in range(NCHUNK):
            qn = persist.tile([128, 4, 385], dt.bfloat16, name=f"qkvn{jc}")
            qkvn.append(qn)
            for t4 in range(4):
                tt = 4 * jc + t4
                ps = pr_ps_pool.tile([128, 384], dt.float32, tag="t", name="qkv_ps")
                for c in range(8):
                    nc.tensor.matmul(
                        ps[:],
                        lhsT=xt_sb[jc][:, c, ts(t4, 128)],
                        rhs=w_sb[:, c, :],
                        start=(c == 0),
                        stop=(c == 7),
                    )
                nc.vector.tensor_copy(qn[:, t4, 0:320], ps[:, 0:320])
                nc.vector.tensor_copy(qn[:, t4, 321:385], ps[:, 320:384])
            nc.gpsimd.memset(qn[:, :, 320:321], 1.0)

            base = qn[:, :, 0:320].rearrange("p f (h d) -> p f h d", h=5)
            x1 = base[:, :, :, 0:32]
            x2 = base[:, :, :, 32:64]
            csc = cs_sb[jc][:]
            cos = csc[:, :, 0:160].rearrange("p f (h d) -> p f h d", h=5)
            sin = csc[:, :, 160:320].rearrange("p f (h d) -> p f h d", h=5)
            t1 = rope_tmp.tile([128, 4, 5, 32], dt.bfloat16, tag="t1")
            t2 = rope_tmp.tile([128, 4, 5, 32], dt.bfloat16, tag="t2")
            t3 = rope_tmp.tile([128, 4, 5, 32], dt.bfloat16, tag="t3")
            t4_ = rope_tmp.tile([128, 4, 5, 32], dt.bfloat16, tag="t4")
            nc.vector.tensor_mul(t1[:], x1, cos)
            nc.vector.tensor_mul(t2[:], x2, sin)
            nc.vector.tensor_mul(t3[:], x1, sin)
            nc.vector.tensor_mul(t4_[:], x2, cos)
            nc.vector.tensor_sub(x1, t1[:], t2[:])
            nc.vector.tensor_add(x2, t3[:], t4_[:])

            kta = persist.tile([128, 4, 128], dt.bfloat16, name=f"kta{jc}")
            ktb = persist.tile([128, 4, 128], dt.bfloat16, name=f"ktb{jc}")
            kt_sb.append((kta, ktb))
            for t4 in range(4):
                nc.sync.dma_start_transpose(
                    qt_sb[0][jc][:, ts(t4, 128)], qn[:, t4, 0:128]
                )
                nc.sync.dma_start_transpose(
                    qt_sb[1][jc][:, ts(t4, 128)], qn[:, t4, 128:256]
                )
                # rows 0-63 <- kT (k dims 256:320); rows 64-127 junk (v cols)
                nc.sync.dma_start_transpose(kta[:, t4, :], qn[:, t4, 256:384])
            # duplicate kT into rows 64-127 of a separate tile so u=0 score
            # matmuls don't wait on this copy (whole-tile dependency)
            nc.gpsimd.dma_start(ktb[64:128, :, :], kta[0:64, :, :])

        # ---- phase 3+4: attention + projection per 512-wide q chunk ----
        # y^T[65, q] = [1|v]^T @ P^T over k tiles; row 0 = softmax denominator.
        # Processed per head-pair so the two y psum regions (2 banks each)
        # double-buffer: pair hp=1 accumulates while hp=0 normalizes.
        for j in range(NCHUNK):
            for hp in range(2):
                y_ps = y_ps_pool.tile(
                    [65, 2, 512], dt.float32, tag=f"y{hp}", name=f"y_ps{hp}"
                )
                for i in range(4 * j + 4):  # k tiles
                    ic, i4 = divmod(i, 4)
                    off = max(0, 128 * i - 512 * j)  # causal: valid q >= 128*i
                    for u in range(2):  # head 2hp+u, kT copy at partitions 64u
                        s_ps = s_ps_pool.tile(
                            [128, 512], dt.float32, tag="s", name="s_ps"
                        )
                        nc.tensor.matmul(
                            s_ps[:, off:512],
                            lhsT=kt_sb[ic][u][ts(u, 64), i4, :],
                            rhs=qt_sb[hp][j][ts(u, 64), off:512],
                            start=True,
                            stop=True,
                        )
                        p_t = p_pool.tile([128, 512], dt.bfloat16, name="p_t")
                        nc.scalar.activation(
                            p_t[:, off:512], s_ps[:, off:512], AF.Exp, scale=0.125
                        )
                        if 128 * i >= 512 * j:  # diagonal block: causal mask
                            # keep where q_local - k_local >= 0, else 0
                            nc.gpsimd.affine_select(
                                p_t[:, off : off + 128],
                                p_t[:, off : off + 128],
                                pattern=[[1, 128]],
                                compare_op=mybir.AluOpType.is_ge,
                                fill=0.0,
                                base=0,
                                channel_multiplier=-1,
                            )
                        nc.tensor.matmul(
                            y_ps[:, u, off:512],
                            lhsT=qkvn[ic][:, i4, 320:385],
                            rhs=p_t[:, off:512],
                            start=(i == 0),
                            stop=(i == 4 * j + 3),
                        )
                # evacuate unnormalized y + den quickly to free the psum banks
                y_sb = ysb_pool.tile([65, 2, 512], dt.float32)
                nc.vector.tensor_copy(y_sb[:], y_ps[:])
                # den row -> reciprocal -> broadcast -> scale
                nc.vector.reciprocal_approx_fast(y_sb[0:1, :, :], y_sb[0:1, :, :])
                bc = bc_pool.tile([65, 2, 512], dt.float32)
                nc.gpsimd.partition_broadcast(bc[:], y_sb[0:1, :, :], channels=65)
                for u in range(2):
                    yst = yst_pool.tile([65, 512], dt.bfloat16)
                    nc.vector.tensor_mul(yst[:, :], y_sb[:, u, :], bc[:, u, :])
                    nc.scalar.dma_start(
                        ynt[hp][j][ts(u, 64), :], yst[1:65, :]
                    )
            # projection for this chunk's 4 token tiles
            for t4 in range(4):
                tt = 4 * j + t4
                po = po_pool.tile([128, 1024], dt.bfloat16)
                for nn2 in range(2):
                    if j == NCHUNK - 1:
                        ps = s_ps_pool.tile(
                            [128, 512], dt.float32, tag="s", name="pr3s_ps"
                        )
                    else:
                        ps = pr_ps_pool.tile(
                            [128, 512], dt.float32, tag="t", name="pr_ps"
                        )
                    for dtile in range(2):
                        nc.tensor.matmul(
                            ps[:],
                            lhsT=ynt[dtile][j][:, ts(t4, 128)],
                            rhs=wo_sb[:, dtile, ts(nn2, 512)],
                            start=(dtile == 0),
                            stop=(dtile == 1),
                        )
                    if j == NCHUNK - 1:
                        nc.scalar.copy(po[:, ts(nn2, 512)], ps[:])
                    else:
                        nc.vector.tensor_copy(po[:, ts(nn2, 512)], ps[:])
                nc.scalar.dma_start(
                    out_d.ap()[ts(tt, 128), :], po[:]
                )

    nc.finalize()
    return nc


def _host_inputs(x, W_qkv, W_proj):
    """Per-core input maps (host-side sharding + partition-major layout)."""
    perm = np.concatenate([np.arange(0, HD, 2), np.arange(1, HD, 2)])  # even|odd
    inv = 1.0 / THETA ** (np.arange(0, HD, 2, dtype=np.float64) / HD)  # [32]
    ang = np.arange(T, dtype=np.float64)[:, None] * inv[None, :]       # [T, 32]
    cos5 = np.tile(np.cos(ang), (1, 5))
    sin5 = np.tile(np.sin(ang), (1, 5))
    cs = np.concatenate([cos5, sin5], axis=1).astype(BF16)             # [T, 320]
    # [T, 320] -> [128, TT*320] partition-major (token t = n*128 + p)
    cs_pm = np.ascontiguousarray(
        cs.reshape(TT, 128, 320).transpose(1, 0, 2).reshape(128, TT * 320)
    )
    masku = np.triu(np.ones((128, 128), dtype=np.float32)).astype(BF16)

    def part_major(a, p=128):  # [R, cols] with R = n*p -> [p, n*cols]
        R, cols = a.shape
        n = R // p
        return np.ascontiguousarray(
            a.reshape(n, p, cols).transpose(1, 0, 2).reshape(p, n * cols)
        )

    in_maps = []
    for core in range(8):
        b, hg = divmod(core, 4)
        xT = x[b].T.astype(BF16)                                       # [C, T]
        # [C, T] -> [128, jc*(8*512)]: (c p), (jc t') -> p, jc, c, t'
        xt_pm = np.ascontiguousarray(
            xT.reshape(8, 128, NCHUNK, 512)
            .transpose(1, 2, 0, 3)
            .reshape(128, NCHUNK * 8 * 512)
        )
        cols = []
        for h in range(hg * NQ, hg * NQ + NQ):
            cols.append(W_qkv[:, h * HD : (h + 1) * HD][:, perm])
        kblk = W_qkv[:, NH * HD + hg * HD : NH * HD + (hg + 1) * HD][:, perm]
        vblk = W_qkv[:, (NH + NKV) * HD + hg * HD : (NH + NKV) * HD + (hg + 1) * HD]
        w384 = np.concatenate(cols + [kblk, vblk], axis=1).astype(BF16)
        wo = W_proj[hg * NQ * HD : (hg + 1) * NQ * HD, :].astype(BF16)
        in_maps.append(
            {
                "xT": xt_pm,
                "w384": part_major(w384),
                "wo": part_major(wo),
                "cs": cs_pm,
                "masku": masku,
            }
        )
    return in_maps


def _run(in_maps):
    from concourse.bass_utils import run_bass_kernel_spmd

    if "nc" not in _CACHE:
        _CACHE["nc"] = _build()
    return run_bass_kernel_spmd(_CACHE["nc"], in_maps, core_ids=list(range(8)))


def kernel(x, W_qkv, W_proj):
    x = np.asarray(x, dtype=np.float32)
    W_qkv = np.asarray(W_qkv, dtype=np.float32)
    W_proj = np.asarray(W_proj, dtype=np.float32)
    res = _run(_host_inputs(x, W_qkv, W_proj))
    out = np.zeros((B, T, C), dtype=np.float32)
    for core in range(8):
        b = core // 4
        out[b] += res.results[core]["out"].astype(np.float32)
    return out


# revision 21
# speedup vs baseline: 780.9537x; 1.2262x over previous
"""Causal GQA self-attention (B=2, T=2048, C=1024, 16 q-heads / 4 kv-heads,
RoPE, causal softmax, output projection) on 8 Trainium2 NeuronCores.

Sharding: core c = b*4 + hg handles batch b (2-way data parallel) and
head-group hg (4-way tensor parallel: its 4 q-heads + their shared kv head).
W_qkv is column-sharded, W_proj row-sharded; each core emits a partial
projection [2048, 1024] and the host sums the 4 partials per batch.

Device pipeline per core (bf16 matmul inputs, fp32 PSUM accumulate):
  1. qkv = x @ W_qkv_shard, natural layout [token, 320|1|64] (q,k | ones | v)
  2. RoPE on q,k in natural layout (head-dim pairs pre-permuted even|odd on
     the host, so the rotation is two contiguous 32-blocks per head)
  3. DMA-xbar-transpose q,k -> qT [256, T]; kT duplicated to partitions 0-63
     and 64-127 (row-packed score matmuls, 2 heads concurrent in the array)
  4. flash-style per 512-wide q chunk: S^T = k @ qT (K=64), exp on ScalarE
     (1/sqrt(64) folded into the activation scale), causal masking only on
     diagonal 128-blocks, y^T[1+64, q] += [1|v]^T @ P^T (v_aug stationary)
  5. y^T row 0 is the softmax denominator: reciprocal_approx + GPSIMD
     partition_broadcast + one fused scale-evacuate multiply
  6. out = yT.T @ W_proj_shard
"""

import sys

if "/opt/trn_rl_repo" not in sys.path:
    sys.path.insert(0, "/opt/trn_rl_repo")

import numpy as np
import ml_dtypes

B, T, C = 2, 2048, 1024
NH, NKV, HD = 16, 4, 64
THETA = 10000.0
NQ = NH // NKV          # q heads per core = 4
TT = T // 128           # 16 token tiles
NCHUNK = T // 512       # 4 q-chunks
BF16 = ml_dtypes.bfloat16

_CACHE = {}


def _build():
    """Build the SPMD Bass program (identical on all 8 cores)."""
    import concourse.mybir as mybir
    import concourse.tile as tile
    from concourse import bacc
    from concourse.bass import ts
    from contextlib import ExitStack

    dt = mybir.dt
    AF = mybir.ActivationFunctionType

    nc = bacc.Bacc("TRN2", target_bir_lowering=False, debug=False, num_devices=8)

    # host pre-shuffled, partition-major inputs (contiguous per partition)
    xt_d = nc.declare_dram_parameter("xT", [128, 8 * T], dt.bfloat16, isOutput=False)
    w_d = nc.declare_dram_parameter("w384", [128, 8 * 384], dt.bfloat16, isOutput=False)
    wo_d = nc.declare_dram_parameter("wo", [128, 2 * C], dt.bfloat16, isOutput=False)
    cs_d = nc.declare_dram_parameter("cs", [128, TT * 320], dt.bfloat16, isOutput=False)
    mk_d = nc.declare_dram_parameter("masku", [128, 128], dt.bfloat16, isOutput=False)
    out_d = nc.declare_dram_parameter("out", [T, C], dt.bfloat16, isOutput=True)

    with tile.TileContext(nc) as tc, ExitStack() as ctx:
        persist = ctx.enter_context(tc.tile_pool(name="persist", bufs=1))
        rope_tmp = ctx.enter_context(tc.tile_pool(name="rope_tmp", bufs=4))
        p_pool = ctx.enter_context(tc.tile_pool(name="p_pool", bufs=14))
        po_pool = ctx.enter_context(tc.tile_pool(name="po", bufs=6))
        ysb_pool = ctx.enter_context(tc.tile_pool(name="ysb", bufs=3))
        bc_pool = ctx.enter_context(tc.tile_pool(name="bc", bufs=2))
        yst_pool = ctx.enter_context(tc.tile_pool(name="yst", bufs=6))
        s_ps_pool = ctx.enter_context(tc.tile_pool(name="s_ps", bufs=3, space="PSUM"))
        pr_ps_pool = ctx.enter_context(tc.tile_pool(name="pr_ps", bufs=1, space="PSUM"))
        y_ps_pool = ctx.enter_context(tc.tile_pool(name="y_ps", bufs=1, space="PSUM"))

        # ---- persistent SBUF (split tiles for fine-grained scheduling) ----
        w_sb = persist.tile([128, 8, 384], dt.bfloat16)
        nc.sync.dma_start(w_sb[:], w_d.ap().rearrange("p (c n) -> p c n", c=8))
        xt_sb = []
        cs_sb = []
        for jc in range(NCHUNK):
            xt = persist.tile([128, 8, 512], dt.bfloat16, name=f"xtc{jc}")
            nc.sync.dma_start(
                xt[:],
                xt_d.ap()[:, ts(jc, 8 * 512)].rearrange("p (c t) -> p c t", c=8),
            )
            xt_sb.append(xt)
            cst = persist.tile([128, 4, 320], dt.bfloat16, name=f"cs{jc}")
            nc.sync.dma_start(
                cst[:],
                cs_d.ap()[:, ts(jc, 4 * 320)].rearrange("p (n d) -> p n d", n=4),
            )
            cs_sb.append(cst)
            if jc == 0:
                mk_sb = persist.tile([128, 128], dt.bfloat16)
                nc.sync.dma_start(mk_sb[:], mk_d.ap())
        wo_sb = persist.tile([128, 2, C], dt.bfloat16)
        nc.sync.dma_start(wo_sb[:], wo_d.ap().rearrange("p (c n) -> p c n", c=2))

        qkvn = []   # per chunk: [128, 4, 385] = q,k rot | ones | v
        qt_sb = [[None] * NCHUNK for _ in range(2)]   # [dimtile][chunk] [128,512]
        kt_sb = []  # per chunk: (ktA rows 0-63, ktB rows 64-127) [128, 4, 128]
        ynt = [[None] * NCHUNK for _ in range(2)]     # [dimtile][chunk] [128,512]
        for d in range(2):
            for j in range(NCHUNK):
                qt_sb[d][j] = persist.tile([128, 512], dt.bfloat16, name=f"qt{d}_{j}")
                ynt[d][j] = persist.tile([128, 512], dt.bfloat16, name=f"ynt{d}_{j}")

        # ---- phase 1+2: qkv, rope, transposes (per 4-token-tile chunk) ----
        for jc in range(NCHUNK):
            qn = persist.tile([128, 4, 385], dt.bfloat16, name=f"qkvn{jc}")
            qkvn.append(qn)
            for t4 in range(4):
                tt = 4 * jc + t4
                if jc == 0:
                    ps = y_ps_pool.tile(
                        [128, 384], dt.float32, tag=f"y{t4 % 2}", name="qkv0_ps"
                    )
                else:
                    ps = pr_ps_pool.tile(
                        [128, 384], dt.float32, tag="t", name="qkv_ps"
                    )
                for c in range(8):
                    nc.tensor.matmul(
                        ps[:],
                        lhsT=xt_sb[jc][:, c, ts(t4, 128)],
                        rhs=w_sb[:, c, :],
                        start=(c == 0),
                        stop=(c == 7),
                    )
                nc.vector.tensor_copy(qn[:, t4, 0:320], ps[:, 0:320])
                nc.vector.tensor_copy(qn[:, t4, 321:385], ps[:, 320:384])
            nc.gpsimd.memset(qn[:, :, 320:321], 1.0)

            base = qn[:, :, 0:320].rearrange("p f (h d) -> p f h d", h=5)
            x1 = base[:, :, :, 0:32]
            x2 = base[:, :, :, 32:64]
            csc = cs_sb[jc][:]
            cos = csc[:, :, 0:160].rearrange("p f (h d) -> p f h d", h=5)
            sin = csc[:, :, 160:320].rearrange("p f (h d) -> p f h d", h=5)
            t1 = rope_tmp.tile([128, 4, 5, 32], dt.bfloat16, tag="t1")
            t2 = rope_tmp.tile([128, 4, 5, 32], dt.bfloat16, tag="t2")
            t3 = rope_tmp.tile([128, 4, 5, 32], dt.bfloat16, tag="t3")
            t4_ = rope_tmp.tile([128, 4, 5, 32], dt.bfloat16, tag="t4")
            nc.vector.tensor_mul(t1[:], x1, cos)
            nc.vector.tensor_mul(t2[:], x2, sin)
            nc.vector.tensor_mul(t3[:], x1, sin)
            nc.vector.tensor_mul(t4_[:], x2, cos)
            nc.vector.tensor_sub(x1, t1[:], t2[:])
            nc.vector.tensor_add(x2, t3[:], t4_[:])

            kta = persist.tile([128, 4, 128], dt.bfloat16, name=f"kta{jc}")
            ktb = persist.tile([128, 4, 128], dt.bfloat16, name=f"ktb{jc}")
            kt_sb.append((kta, ktb))
            for t4 in range(4):
                nc.sync.dma_start_transpose(
                    qt_sb[0][jc][:, ts(t4, 128)], qn[:, t4, 0:128]
                )
                nc.sync.dma_start_transpose(
                    qt_sb[1][jc][:, ts(t4, 128)], qn[:, t4, 128:256]
                )
                # rows 0-63 <- kT (k dims 256:320); rows 64-127 junk (v cols)
                nc.sync.dma_start_transpose(kta[:, t4, :], qn[:, t4, 256:384])
            # duplicate kT into rows 64-127 of a separate tile so u=0 score
            # matmuls don't wait on this copy (whole-tile dependency)
            nc.gpsimd.dma_start(ktb[64:128, :, :], kta[0:64, :, :])

        # ---- phase 3+4: attention + projection per 512-wide q chunk ----
        # y^T[65, q] = [1|v]^T @ P^T over k tiles; row 0 = softmax denominator.
        # Processed per head-pair so the two y psum regions (2 banks each)
        # double-buffer: pair hp=1 accumulates while hp=0 normalizes.
        for j in range(NCHUNK):
            for hp in range(2):
                y_ps = y_ps_pool.tile(
                    [65, 2, 512], dt.float32, tag=f"y{hp}", name=f"y_ps{hp}"
                )
                for i in range(4 * j + 4):  # k tiles
                    ic, i4 = divmod(i, 4)
                    off = max(0, 128 * i - 512 * j)  # causal: valid q >= 128*i
                    for u in range(2):  # head 2hp+u, kT copy at partitions 64u
                        s_ps = s_ps_pool.tile(
                            [128, 512], dt.float32, tag="s", name="s_ps"
                        )
                        nc.tensor.matmul(
                            s_ps[:, off:512],
                            lhsT=kt_sb[ic][u][ts(u, 64), i4, :],
                            rhs=qt_sb[hp][j][ts(u, 64), off:512],
                            start=True,
                            stop=True,
                        )
                        p_t = p_pool.tile([128, 512], dt.bfloat16, name="p_t")
                        nc.scalar.activation(
                            p_t[:, off:512], s_ps[:, off:512], AF.Exp, scale=0.125
                        )
                        if 128 * i >= 512 * j:  # diagonal block: causal mask
                            # keep where q_local - k_local >= 0, else 0
                            nc.gpsimd.affine_select(
                                p_t[:, off : off + 128],
                                p_t[:, off : off + 128],
                                pattern=[[1, 128]],
                                compare_op=mybir.AluOpType.is_ge,
                                fill=0.0,
                                base=0,
                                channel_multiplier=-1,
                            )
                        nc.tensor.matmul(
                            y_ps[:, u, off:512],
                            lhsT=qkvn[ic][:, i4, 320:385],
                            rhs=p_t[:, off:512],
                            start=(i == 0),
                            stop=(i == 4 * j + 3),
                        )
                # evacuate unnormalized y + den quickly to free the psum banks
                y_sb = ysb_pool.tile([65, 2, 512], dt.float32)
                nc.vector.tensor_copy(y_sb[:], y_ps[:])
                # den row -> reciprocal -> broadcast -> scale
                nc.vector.reciprocal_approx_fast(y_sb[0:1, :, :], y_sb[0:1, :, :])
                bc = bc_pool.tile([65, 2, 512], dt.float32)
                nc.gpsimd.partition_broadcast(bc[:], y_sb[0:1, :, :], channels=65)
                for u in range(2):
                    yst = yst_pool.tile([65, 512], dt.bfloat16)
                    nc.vector.tensor_mul(yst[:, :], y_sb[:, u, :], bc[:, u, :])
                    nc.scalar.dma_start(
                        ynt[hp][j][ts(u, 64), :], yst[1:65, :]
                    )
            # projection for this chunk's 4 token tiles
            for t4 in range(4):
                tt = 4 * j + t4
                po = po_pool.tile([128, 1024], dt.bfloat16)
                for nn2 in range(2):
                    if j == NCHUNK - 1:
                        ps = s_ps_pool.tile(
                            [128, 512], dt.float32, tag="s", name="pr3s_ps"
                        )
                    else:
                        ps = pr_ps_pool.tile(
                            [128, 512], dt.float32, tag="t", name="pr_ps"
                        )
                    for dtile in range(2):
                        nc.tensor.matmul(
                            ps[:],
                            lhsT=ynt[dtile][j][:, ts(t4, 128)],
                            rhs=wo_sb[:, dtile, ts(nn2, 512)],
                            start=(dtile == 0),
                            stop=(dtile == 1),
                        )
                    if j == NCHUNK - 1:
                        nc.scalar.copy(po[:, ts(nn2, 512)], ps[:])
                    else:
                        nc.vector.tensor_copy(po[:, ts(nn2, 512)], ps[:])
                nc.scalar.dma_start(
                    out_d.ap()[ts(tt, 128), :], po[:]
                )

    nc.finalize()
    return nc


def _host_inputs(x, W_qkv, W_proj):
    """Per-core input maps (host-side sharding + partition-major layout)."""
    perm = np.concatenate([np.arange(0, HD, 2), np.arange(1, HD, 2)])  # even|odd
    inv = 1.0 / THETA ** (np.arange(0, HD, 2, dtype=np.float64) / HD)  # [32]
    ang = np.arange(T, dtype=np.float64)[:, None] * inv[None, :]       # [T, 32]
    cos5 = np.tile(np.cos(ang), (1, 5))
    sin5 = np.tile(np.sin(ang), (1, 5))
    cs = np.concatenate([cos5, sin5], axis=1).astype(BF16)             # [T, 320]
    # [T, 320] -> [128, TT*320] partition-major (token t = n*128 + p)
    cs_pm = np.ascontiguousarray(
        cs.reshape(TT, 128, 320).transpose(1, 0, 2).reshape(128, TT * 320)
    )
    masku = np.triu(np.ones((128, 128), dtype=np.float32)).astype(BF16)

    def part_major(a, p=128):  # [R, cols] with R = n*p -> [p, n*cols]
        R, cols = a.shape
        n = R // p
        return np.ascontiguousarray(
            a.reshape(n, p, cols).transpose(1, 0, 2).reshape(p, n * cols)
        )

    in_maps = []
    for core in range(8):
        b, hg = divmod(core, 4)
        xT = x[b].T.astype(BF16)                                       # [C, T]
        # [C, T] -> [128, jc*(8*512)]: (c p), (jc t') -> p, jc, c, t'
        xt_pm = np.ascontiguousarray(
            xT.reshape(8, 128, NCHUNK, 512)
            .transpose(1, 2, 0, 3)
            .reshape(128, NCHUNK * 8 * 512)
        )
        cols = []
        for h in range(hg * NQ, hg * NQ + NQ):
            cols.append(W_qkv[:, h * HD : (h + 1) * HD][:, perm])
        kblk = W_qkv[:, NH * HD + hg * HD : NH * HD + (hg + 1) * HD][:, perm]
        vblk = W_qkv[:, (NH + NKV) * HD + hg * HD : (NH + NKV) * HD + (hg + 1) * HD]
        w384 = np.concatenate(cols + [kblk, vblk], axis=1).astype(BF16)
        wo = W_proj[hg * NQ * HD : (hg + 1) * NQ * HD, :].astype(BF16)
        in_maps.append(
            {
                "xT": xt_pm,
                "w384": part_major(w384),
                "wo": part_major(wo),
                "cs": cs_pm,
                "masku": masku,
            }
        )
    return in_maps


def _run(in_maps):
    from concourse.bass_utils import run_bass_kernel_spmd

    if "nc" not in _CACHE:
        _CACHE["nc"] = _build()
    return run_bass_kernel_spmd(_CACHE["nc"], in_maps, core_ids=list(range(8)))


def kernel(x, W_qkv, W_proj):
    x = np.asarray(x, dtype=np.float32)
    W_qkv = np.asarray(W_qkv, dtype=np.float32)
    W_proj = np.asarray(W_proj, dtype=np.float32)
    res = _run(_host_inputs(x, W_qkv, W_proj))
    out = np.zeros((B, T, C), dtype=np.float32)
    for core in range(8):
        b = core // 4
        out[b] += res.results[core]["out"].astype(np.float32)
    return out
